# revision 1
# baseline (speedup 1.0000x reference)
"""Trainium2 Bass kernel for nn_DistillMoE (noisy top-2 MoE, 8 experts).

Strategy: data-parallel over 8 NeuronCores (16384 tokens each).
Per core: PE-transpose x tiles (fp32, exact) -> fp32 router matmuls ->
noisy-top2 gating on DVE/ACT -> dense gate-weighted expert matmuls in
float32r (full-rate, ~1e-4) -> combine on DVE.

kernel(**inputs) takes the FULL inputs and returns (updates, gating_output)
exactly like the reference nn.Module.
"""
import sys
import numpy as np

sys.path.insert(0, "/opt/trn_rl_repo")

from concourse import bacc, mybir  # noqa: E402
from concourse.tile import TileContext  # noqa: E402
from concourse.bass_utils import run_bass_kernel_spmd  # noqa: E402
from concourse.masks import make_identity  # noqa: E402

N_CORES = 8
B_FULL = 131072
D = 512
E = 8
BC = B_FULL // N_CORES          # tokens per core
N_TILES = BC // 128             # 128
ST = 4                          # tiles per super-tile
N_ST = N_TILES // ST            # 32

f32 = mybir.dt.float32
f32r = mybir.dt.float32r
AF = mybir.ActivationFunctionType
OP = mybir.AluOpType

_CACHE = {}


def _build(use_bias: bool, use_rbias: bool):
    key = (use_bias, use_rbias)
    if key in _CACHE:
        return _CACHE[key]
    nc = bacc.Bacc("TRN2", target_bir_lowering=False, debug=False, num_devices=N_CORES)

    d_x = nc.dram_tensor("x", [BC, D], f32, kind="ExternalInput")
    d_noise = nc.dram_tensor("noise", [BC, E], f32, kind="ExternalInput")
    d_wrn = nc.dram_tensor("wrn", [D, 2 * E], f32, kind="ExternalInput")
    d_brn = nc.dram_tensor("brn", [1, 2 * E], f32, kind="ExternalInput")
    d_we = nc.dram_tensor("we", [E, D, D], f32r, kind="ExternalInput")
    d_be = nc.dram_tensor("be", [E, D], f32r, kind="ExternalInput")

    d_out = nc.dram_tensor("out", [BC, D], f32, kind="ExternalOutput")
    d_gat = nc.dram_tensor("gat", [BC, E], f32, kind="ExternalOutput")

    x_t4 = d_x.ap().rearrange("(s j p) d -> s j p d", p=128, j=ST)
    out_t4 = d_out.ap().rearrange("(s j p) d -> s j p d", p=128, j=ST)
    noise_st = d_noise.ap().rearrange("(s j p) e -> s p j e", p=128, j=ST)
    gat_st_d = d_gat.ap().rearrange("(s j p) e -> s p j e", p=128, j=ST)

    with TileContext(nc) as tc:
        with tc.tile_pool(name="const", bufs=1) as cpool, \
             tc.tile_pool(name="wpool", bufs=1) as wpool, \
             tc.tile_pool(name="sb", bufs=3) as sb, \
             tc.tile_pool(name="st", bufs=2) as stp, \
             tc.tile_pool(name="ps", bufs=2, space="PSUM") as ps:

            ident = cpool.tile([128, 128], f32, tag="ident")
            make_identity(nc, ident[:])
            wrn_sb = cpool.tile([128, 4, 2 * E], f32, tag="wrn")
            nc.sync.dma_start(wrn_sb[:], d_wrn.ap().rearrange("(c p) n -> p c n", p=128))
            we_sb = wpool.tile([128, E, 4, D], f32r, tag="we")
            nc.sync.dma_start(we_sb[:], d_we.ap().rearrange("e (c p) n -> p e c n", p=128))
            if use_rbias:
                brn_sb = cpool.tile([1, 2 * E], f32, tag="brn")
                nc.sync.dma_start(brn_sb[:], d_brn.ap())
            if use_bias or use_rbias:
                ones_r = cpool.tile([1, 128], f32r, tag="ones")
                nc.vector.memset(ones_r[:], 1.0)
                ones_f = cpool.tile([1, 128], f32, tag="onesf")
                nc.vector.memset(ones_f[:], 1.0)
            if use_bias:
                be_sb = cpool.tile([1, E, D], f32r, tag="be")
                nc.sync.dma_start(be_sb[:], d_be.ap()[None])

            for s in range(N_ST):
                xt32 = stp.tile([128, ST, 4, 128], f32, tag="xt32")
                xtr = stp.tile([128, ST, 4, 128], f32r, tag="xtr")
                lg = stp.tile([128, ST, 2 * E], f32, tag="lg")

                for j in range(ST):
                    x_t = sb.tile([128, D], f32, tag="x")
                    nc.sync.dma_start(x_t[:], x_t4[s, j])
                    for c in range(4):
                        tp = ps.tile([128, 128], f32, tag="tp")
                        nc.tensor.transpose(tp[:], x_t[:, c * 128:(c + 1) * 128], ident[:])
                        nc.vector.tensor_copy(xt32[:, j, c], tp[:])
                        nc.scalar.activation(xtr[:, j, c], tp[:], AF.Copy)
                    psr = ps.tile([128, 2 * E], f32, tag="psr")
                    for c in range(4):
                        nc.tensor.matmul(psr[:], xt32[:, j, c], wrn_sb[:, c],
                                         start=(c == 0), stop=(c == 3 and not use_rbias))
                    if use_rbias:
                        nc.tensor.matmul(psr[:], ones_f[:], brn_sb[:], start=False, stop=True)
                    nc.vector.tensor_copy(lg[:, j], psr[:])

                # ---- routing vector stage on [128, ST, 8] ----
                noise_t = sb.tile([128, ST, E], f32, tag="noise")
                nc.sync.dma_start(noise_t[:], noise_st[s])
                logit = lg[:, :, 0:E]
                nlog = lg[:, :, E:2 * E]
                shp = (128, ST, E)
                t_a = sb.tile(list(shp), f32, tag="va")
                nc.scalar.activation(t_a[:], nlog, AF.Abs)
                nc.scalar.activation(t_a[:], t_a[:], AF.Exp, scale=-1.0)
                nc.scalar.activation(t_a[:], t_a[:], AF.Ln, bias=1.0)
                # softplus = max(nlog,0) + ln(1+exp(-|nlog|))
                nc.vector.scalar_tensor_tensor(t_a[:], nlog, 0.0, t_a[:], OP.max, OP.add)
                noisy = sb.tile(list(shp), f32, tag="vn")
                nc.vector.tensor_tensor(noisy[:], noise_t[:], t_a[:], OP.mult)
                nc.vector.tensor_tensor(noisy[:], noisy[:], logit, OP.add)
                m1 = sb.tile([128, ST, 1], f32, tag="vm1")
                nc.vector.tensor_reduce(m1[:], noisy[:], mybir.AxisListType.X, OP.max)
                eq = sb.tile(list(shp), f32, tag="veq")
                nc.vector.tensor_tensor(eq[:], noisy[:], m1[:].to_broadcast(shp), OP.is_equal)
                nc.vector.scalar_tensor_tensor(eq[:], eq[:], -1e30, noisy[:], OP.mult, OP.add)
                m2 = sb.tile([128, ST, 1], f32, tag="vm2")
                nc.vector.tensor_reduce(m2[:], eq[:], mybir.AxisListType.X, OP.max)
                mask2 = sb.tile(list(shp), f32, tag="vmk")
                nc.vector.tensor_tensor(mask2[:], noisy[:], m2[:].to_broadcast(shp), OP.is_ge)
                sh = sb.tile(list(shp), f32, tag="vsh")
                nc.vector.tensor_tensor(sh[:], noisy[:], m1[:].to_broadcast(shp), OP.subtract)
                nc.scalar.activation(sh[:], sh[:], AF.Exp)
                gat = stp.tile([128, ST, E], f32, tag="gat")
                nc.vector.tensor_tensor(gat[:], sh[:], mask2[:], OP.mult)
                den = sb.tile([128, ST, 1], f32, tag="vdn")
                nc.vector.tensor_reduce(den[:], gat[:], mybir.AxisListType.X, OP.add)
                nc.vector.reciprocal(den[:], den[:])
                nc.vector.tensor_tensor(gat[:], gat[:], den[:].to_broadcast(shp), OP.mult)
                nc.sync.dma_start(gat_st_d[s], gat[:])

                # ---- dense gated experts ----
                for j in range(ST):
                    acc = sb.tile([128, D], f32, tag="acc")
                    for e in range(E):
                        pse = ps.tile([128, D], f32, tag="pse")
                        for c in range(4):
                            nc.tensor.matmul(pse[:], xtr[:, j, c], we_sb[:, e, c],
                                             start=(c == 0), stop=(c == 3 and not use_bias))
                        if use_bias:
                            nc.tensor.matmul(pse[:], ones_r[:], be_sb[:, e], start=False, stop=True)
                        g = gat[:, j, e:e + 1]
                        if e == 0:
                            nc.vector.tensor_scalar_mul(acc[:], pse[:], g)
                        else:
                            nc.vector.scalar_tensor_tensor(acc[:], pse[:], g, acc[:], OP.mult, OP.add)
                    nc.sync.dma_start(out_t4[s, j], acc[:])

    nc.compile()
    _CACHE[key] = nc
    return nc


def _compute_noise():
    import jax
    cpu = jax.devices("cpu")[0]
    with jax.default_device(cpu):
        import jax.numpy as jnp
        key = jax.random.key(1234)
        return np.asarray(jax.random.normal(key, (B_FULL, E), dtype=jnp.float32))


def kernel(x, Wr, br, Wn, bn, We, be):
    x = np.ascontiguousarray(np.asarray(x, dtype=np.float32))
    Wr = np.asarray(Wr, dtype=np.float32)
    br = np.asarray(br, dtype=np.float32)
    Wn = np.asarray(Wn, dtype=np.float32)
    bn = np.asarray(bn, dtype=np.float32)
    We = np.ascontiguousarray(np.asarray(We, dtype=np.float32))
    be = np.ascontiguousarray(np.asarray(be, dtype=np.float32))

    noise = _compute_noise()
    wrn = np.ascontiguousarray(np.concatenate([Wr, Wn], axis=1))       # [512, 16]
    brn = np.concatenate([br, bn])[None, :]                            # [1, 16]
    use_bias = bool(np.any(be != 0.0))
    use_rbias = bool(np.any(brn != 0.0))

    nc = _build(use_bias, use_rbias)

    in_maps = []
    for c in range(N_CORES):
        sl = slice(c * BC, (c + 1) * BC)
        in_maps.append({
            "x": x[sl],
            "noise": noise[sl],
            "wrn": wrn,
            "brn": brn,
            "we": We,
            "be": be,
        })
    res = run_bass_kernel_spmd(nc, in_maps, core_ids=list(range(N_CORES)))
    updates = np.concatenate([r["out"] for r in res.results], axis=0)
    gating = np.concatenate([r["gat"] for r in res.results], axis=0)
    return updates, gating


if __name__ == "__main__":
    rng = np.random.default_rng(0)
    print("building...")
    _build(False, False)
    print("built ok")


# revision 2
# speedup vs baseline: 1.1311x; 1.1311x over previous
"""Trainium2 Bass kernel for nn_DistillMoE (noisy top-2 MoE, 8 experts, B=131072, D=512).

Strategy (v2, sparse dispatch):
- 8-way data parallel. The host assigns tokens to cores grouped by their
  top-2 expert PAIR (28 unordered pairs), round-robin per pair so every
  core gets an identical static slot schedule (group -> slot-tile map is
  baked into the traced program; all data-dependence flows through an
  int32 permutation tensor consumed by indirect DMA).
- Device pass 1 (per core): PE-transpose x tiles (fp32 exact) -> fp32
  router matmuls -> full noisy-top2 softmax gating on DVE/ACT (device
  computes ALL routing math; host pair-assignment only affects
  scheduling, and host/device agree except on ~1e-7 ties).
- Device pass 2: for each 128-slot tile (expert pair (a,b) static):
  indirect-gather x rows + gating rows, PE-transpose, two float32r
  expert GEMMs (full PE rate, ~1e-4 accuracy), gate-combine on ACT+DVE,
  indirect-scatter to the output.

kernel(**inputs) takes FULL inputs, returns (updates, gating_output).
"""
import sys
import numpy as np

sys.path.insert(0, "/opt/trn_rl_repo")

from concourse import bacc, mybir  # noqa: E402
from concourse import bass  # noqa: E402
from concourse.tile import TileContext  # noqa: E402
from concourse.bass_utils import run_bass_kernel_spmd  # noqa: E402
from concourse.masks import make_identity  # noqa: E402

N_CORES = 8
B_FULL = 131072
D = 512
E = 8
SH = 16896                      # padded per-core shard rows (132 tiles)
NT1 = SH // 128                 # 132 pass-1 tiles
ST = 4                          # tiles per super-tile
N_ST = NT1 // ST                # 33
SLACK = 16                      # per-group capacity slack (tokens)

f32 = mybir.dt.float32
f32r = mybir.dt.float32r
i32 = mybir.dt.int32
AF = mybir.ActivationFunctionType
OP = mybir.AluOpType

_CACHE = {}


def _build(schedule, use_bias, use_rbias):
    """schedule: tuple of (a, b) expert pairs, one per 128-slot pass-2 tile."""
    key = (schedule, use_bias, use_rbias)
    if key in _CACHE:
        return _CACHE[key]
    NT2 = len(schedule)
    nc = bacc.Bacc("TRN2", target_bir_lowering=False, debug=False, num_devices=N_CORES)

    d_x = nc.dram_tensor("x", [SH, D], f32, kind="ExternalInput")
    d_noise = nc.dram_tensor("noise", [SH, E], f32, kind="ExternalInput")
    d_wrn = nc.dram_tensor("wrn", [D, 2 * E], f32, kind="ExternalInput")
    d_brn = nc.dram_tensor("brn", [1, 2 * E], f32, kind="ExternalInput")
    d_we = nc.dram_tensor("we", [E, D, D], f32r, kind="ExternalInput")
    d_be = nc.dram_tensor("be", [E, D], f32r, kind="ExternalInput")
    d_perm = nc.dram_tensor("perm", [NT2, 128, 1], i32, kind="ExternalInput")

    d_out = nc.dram_tensor("out", [SH, D], f32, kind="ExternalOutput")
    d_gat = nc.dram_tensor("gat", [SH, E], f32, kind="ExternalOutput")

    x_t4 = d_x.ap().rearrange("(s j p) d -> s j p d", p=128, j=ST)
    noise_st = d_noise.ap().rearrange("(s j p) e -> s p j e", p=128, j=ST)
    gat_st_d = d_gat.ap().rearrange("(s j p) e -> s p j e", p=128, j=ST)

    with TileContext(nc) as tc:
        with tc.tile_pool(name="const", bufs=1) as cpool, \
             tc.tile_pool(name="wpool", bufs=1) as wpool, \
             tc.tile_pool(name="sb", bufs=3) as sb, \
             tc.tile_pool(name="st", bufs=2) as stp, \
             tc.tile_pool(name="p2", bufs=3) as p2, \
             tc.tile_pool(name="dr", bufs=1, space="DRAM") as drp, \
             tc.tile_pool(name="ps", bufs=2, space="PSUM") as ps, \
             tc.tile_pool(name="pse", bufs=4, space="PSUM") as pse_pool:

            ident = cpool.tile([128, 128], f32, tag="ident")
            make_identity(nc, ident[:])
            wrn_sb = cpool.tile([128, 4, 2 * E], f32, tag="wrn")
            nc.sync.dma_start(wrn_sb[:], d_wrn.ap().rearrange("(c p) n -> p c n", p=128))
            we_sb = wpool.tile([128, E, 4, D], f32r, tag="we")
            nc.sync.dma_start(we_sb[:], d_we.ap().rearrange("e (c p) n -> p e c n", p=128))
            if use_rbias:
                brn_sb = cpool.tile([1, 2 * E], f32, tag="brn")
                nc.sync.dma_start(brn_sb[:], d_brn.ap())
                ones_f = cpool.tile([1, 128], f32, tag="onesf")
                nc.vector.memset(ones_f[:], 1.0)
            if use_bias:
                ones_r = cpool.tile([1, 128], f32r, tag="ones")
                nc.vector.memset(ones_r[:], 1.0)
                be_sb = cpool.tile([1, E, D], f32r, tag="be")
                nc.sync.dma_start(be_sb[:], d_be.ap()[None])

            gat_scr = drp.tile([SH, E], f32, tag="gscr")
            gat_scr_st = gat_scr[:].rearrange("(s j p) e -> s p j e", p=128, j=ST)

            # ---------------- pass 1: router + gating ----------------
            for s in range(N_ST):
                xt32 = stp.tile([128, ST, 4, 128], f32, tag="xt32")
                lg = stp.tile([128, ST, 2 * E], f32, tag="lg")

                for j in range(ST):
                    x_t = sb.tile([128, D], f32, tag="x")
                    nc.sync.dma_start(x_t[:], x_t4[s, j])
                    for c in range(4):
                        tp = ps.tile([128, 128], f32, tag="tp")
                        nc.tensor.transpose(tp[:], x_t[:, c * 128:(c + 1) * 128], ident[:])
                        nc.vector.tensor_copy(xt32[:, j, c], tp[:])
                    psr = ps.tile([128, 2 * E], f32, tag="psr")
                    for c in range(4):
                        nc.tensor.matmul(psr[:], xt32[:, j, c], wrn_sb[:, c],
                                         start=(c == 0), stop=(c == 3 and not use_rbias))
                    if use_rbias:
                        nc.tensor.matmul(psr[:], ones_f[:], brn_sb[:], start=False, stop=True)
                    nc.vector.tensor_copy(lg[:, j], psr[:])

                noise_t = sb.tile([128, ST, E], f32, tag="noise")
                nc.sync.dma_start(noise_t[:], noise_st[s])
                logit = lg[:, :, 0:E]
                nlog = lg[:, :, E:2 * E]
                shp = (128, ST, E)
                t_a = sb.tile(list(shp), f32, tag="va")
                nc.scalar.activation(t_a[:], nlog, AF.Abs)
                nc.scalar.activation(t_a[:], t_a[:], AF.Exp, scale=-1.0)
                nc.scalar.activation(t_a[:], t_a[:], AF.Ln, bias=1.0)
                nc.vector.scalar_tensor_tensor(t_a[:], nlog, 0.0, t_a[:], OP.max, OP.add)
                noisy = sb.tile(list(shp), f32, tag="vn")
                nc.vector.tensor_tensor(noisy[:], noise_t[:], t_a[:], OP.mult)
                nc.vector.tensor_tensor(noisy[:], noisy[:], logit, OP.add)
                m1 = sb.tile([128, ST, 1], f32, tag="vm1")
                nc.vector.tensor_reduce(m1[:], noisy[:], mybir.AxisListType.X, OP.max)
                eq = sb.tile(list(shp), f32, tag="veq")
                nc.vector.tensor_tensor(eq[:], noisy[:], m1[:].to_broadcast(shp), OP.is_equal)
                nc.vector.scalar_tensor_tensor(eq[:], eq[:], -1e30, noisy[:], OP.mult, OP.add)
                m2 = sb.tile([128, ST, 1], f32, tag="vm2")
                nc.vector.tensor_reduce(m2[:], eq[:], mybir.AxisListType.X, OP.max)
                mask2 = sb.tile(list(shp), f32, tag="vmk")
                nc.vector.tensor_tensor(mask2[:], noisy[:], m2[:].to_broadcast(shp), OP.is_ge)
                sh_t = sb.tile(list(shp), f32, tag="vsh")
                nc.vector.tensor_tensor(sh_t[:], noisy[:], m1[:].to_broadcast(shp), OP.subtract)
                nc.scalar.activation(sh_t[:], sh_t[:], AF.Exp)
                gat = stp.tile([128, ST, E], f32, tag="gat")
                nc.vector.tensor_tensor(gat[:], sh_t[:], mask2[:], OP.mult)
                den = sb.tile([128, ST, 1], f32, tag="vdn")
                nc.vector.tensor_reduce(den[:], gat[:], mybir.AxisListType.X, OP.add)
                nc.vector.reciprocal(den[:], den[:])
                nc.vector.tensor_tensor(gat[:], gat[:], den[:].to_broadcast(shp), OP.mult)
                nc.sync.dma_start(gat_st_d[s], gat[:])
                nc.sync.dma_start(gat_scr_st[s], gat[:])

            # ---------------- pass 2: sparse expert GEMMs ----------------
            for t in range(NT2):
                ea, eb = schedule[t]
                perm_t = p2.tile([128, 1], i32, tag="perm")
                nc.sync.dma_start(perm_t[:], d_perm.ap()[t])
                xg = p2.tile([128, D], f32, tag="xg")
                nc.gpsimd.indirect_dma_start(
                    out=xg[:], out_offset=None, in_=d_x.ap(),
                    in_offset=bass.IndirectOffsetOnAxis(ap=perm_t[:, :1], axis=0))
                gatr = p2.tile([128, E], f32, tag="gatr")
                nc.gpsimd.indirect_dma_start(
                    out=gatr[:], out_offset=None, in_=gat_scr[:],
                    in_offset=bass.IndirectOffsetOnAxis(ap=perm_t[:, :1], axis=0))

                xtr = p2.tile([128, 4, 128], f32r, tag="xtr")
                for c in range(4):
                    tp = ps.tile([128, 128], f32, tag="tp")
                    nc.tensor.transpose(tp[:], xg[:, c * 128:(c + 1) * 128], ident[:])
                    if c % 2 == 0:
                        nc.vector.tensor_copy(xtr[:, c], tp[:])
                    else:
                        nc.scalar.activation(xtr[:, c], tp[:], AF.Copy)

                ps_a = pse_pool.tile([128, D], f32, tag="pse")
                ps_b = pse_pool.tile([128, D], f32, tag="pse")
                for c in range(4):
                    nc.tensor.matmul(ps_a[:], xtr[:, c], we_sb[:, ea, c],
                                     start=(c == 0), stop=(c == 3 and not use_bias))
                for c in range(4):
                    nc.tensor.matmul(ps_b[:], xtr[:, c], we_sb[:, eb, c],
                                     start=(c == 0), stop=(c == 3 and not use_bias))
                if use_bias:
                    nc.tensor.matmul(ps_a[:], ones_r[:], be_sb[:, ea], start=False, stop=True)
                    nc.tensor.matmul(ps_b[:], ones_r[:], be_sb[:, eb], start=False, stop=True)

                acc = p2.tile([128, D], f32, tag="acc")
                nc.scalar.activation(acc[:], ps_a[:], AF.Copy, scale=gatr[:, ea:ea + 1])
                nc.vector.scalar_tensor_tensor(acc[:], ps_b[:], gatr[:, eb:eb + 1], acc[:],
                                               OP.mult, OP.add)
                nc.gpsimd.indirect_dma_start(
                    out=d_out.ap(), out_offset=bass.IndirectOffsetOnAxis(ap=perm_t[:, :1], axis=0),
                    in_=acc[:], in_offset=None)

    nc.compile()
    _CACHE[key] = nc
    return nc


def _compute_noise():
    import jax
    cpu = jax.devices("cpu")[0]
    with jax.default_device(cpu):
        import jax.numpy as jnp
        key = jax.random.key(1234)
        return np.asarray(jax.random.normal(key, (B_FULL, E), dtype=jnp.float32))


def _route_host(x, Wr, br, Wn, bn, noise):
    """Replicate the reference routing decisions (fp32 numpy)."""
    logits = x @ Wr + br
    nl = x @ Wn + bn
    noisy = logits + noise * np.logaddexp(nl, 0.0).astype(np.float32)
    order = np.argsort(-noisy, axis=1, kind="stable")
    e1, e2 = order[:, 0].astype(np.int32), order[:, 1].astype(np.int32)
    a = np.minimum(e1, e2)
    b = np.maximum(e1, e2)
    return a * 8 + b


def kernel(x, Wr, br, Wn, bn, We, be):
    x = np.ascontiguousarray(np.asarray(x, dtype=np.float32))
    Wr = np.asarray(Wr, dtype=np.float32)
    br = np.asarray(br, dtype=np.float32)
    Wn = np.asarray(Wn, dtype=np.float32)
    bn = np.asarray(bn, dtype=np.float32)
    We = np.ascontiguousarray(np.asarray(We, dtype=np.float32))
    be = np.ascontiguousarray(np.asarray(be, dtype=np.float32))

    noise = _compute_noise()
    gid = _route_host(x, Wr, br, Wn, bn, noise)

    # group tokens by expert pair; round-robin each pair's tokens over cores
    groups = []           # (gid, token_array)
    for g in range(64):
        idxs = np.nonzero(gid == g)[0]
        if len(idxs):
            groups.append((g, idxs))

    caps = []
    for g, idxs in groups:
        per_core_max = -(-len(idxs) // N_CORES)
        caps.append(-(-(per_core_max + SLACK) // 128) * 128)
    NT2 = sum(caps) // 128
    schedule = []
    for (g, _), cap in zip(groups, caps):
        schedule.extend([(g // 8, g % 8)] * (cap // 128))
    schedule = tuple(schedule)

    wrn = np.ascontiguousarray(np.concatenate([Wr, Wn], axis=1))
    brn = np.concatenate([br, bn])[None, :]
    use_bias = bool(np.any(be != 0.0))
    use_rbias = bool(np.any(brn != 0.0))

    nc = _build(schedule, use_bias, use_rbias)

    in_maps = []
    tok_lists = []
    for c in range(N_CORES):
        tok_list = np.concatenate([idxs[c::N_CORES] for _, idxs in groups])
        assert len(tok_list) <= SH - 1
        tok_lists.append(tok_list)
        x_c = np.zeros((SH, D), np.float32)
        x_c[:len(tok_list)] = x[tok_list]
        n_c = np.zeros((SH, E), np.float32)
        n_c[:len(tok_list)] = noise[tok_list]
        perm = np.full((NT2 * 128,), SH - 1, np.int32)
        off = 0
        loc = 0
        for (g, idxs), cap in zip(groups, caps):
            n_g = len(idxs[c::N_CORES])
            perm[off:off + n_g] = np.arange(loc, loc + n_g, dtype=np.int32)
            off += cap
            loc += n_g
        in_maps.append({
            "x": x_c, "noise": n_c, "wrn": wrn, "brn": brn,
            "we": We, "be": be, "perm": perm.reshape(NT2, 128, 1),
        })

    res = run_bass_kernel_spmd(nc, in_maps, core_ids=list(range(N_CORES)))
    updates = np.empty((B_FULL, D), np.float32)
    gating = np.empty((B_FULL, E), np.float32)
    for c in range(N_CORES):
        tl = tok_lists[c]
        updates[tl] = res.results[c]["out"][:len(tl)]
        gating[tl] = res.results[c]["gat"][:len(tl)]
    return updates, gating


if __name__ == "__main__":
    print("smoke build with a fake schedule...")
    sched = tuple((g // 8, g % 8) for g in range(28))
    _build(sched, False, False)
    print("built ok")


# revision 5
# speedup vs baseline: 1.6855x; 1.4901x over previous
"""Trainium2 Bass kernel for nn_DistillMoE (noisy top-2 MoE, 8 experts, B=131072, D=512).

Strategy (v3, host-sorted sparse dispatch, fully fused):
- 8-way data parallel over NeuronCores. The host groups tokens by their
  top-2 expert PAIR (28 unordered pairs) and round-robins each pair's
  tokens across cores, so all cores share ONE static slot schedule.
  Each core's input shard is laid out in slot order (group regions
  128-aligned, zero padding rows) — so the device needs NO gather or
  scatter at all; the expert pair for each 128-token tile is baked into
  the traced program.
- Device, per 128-token tile: PE-transpose (fp32, exact) -> fp32 router
  matmuls -> noisy-top2 softmax gating on DVE/ACT (all routing math on
  device) -> TWO float32r expert GEMMs (full PE rate, ~1e-4) -> gate
  combine on ACT+DVE -> stream out. Host scatters rows back to the
  original token order.

kernel(**inputs) takes FULL inputs, returns (updates, gating_output).
"""
import sys
import numpy as np

sys.path.insert(0, "/opt/trn_rl_repo")

from concourse import bacc, mybir  # noqa: E402
from concourse.tile import TileContext  # noqa: E402
from concourse.bass_utils import run_bass_kernel_spmd  # noqa: E402
from concourse.masks import make_identity  # noqa: E402

N_CORES = 8
B_FULL = 131072
D = 512
E = 8
ST = 4                          # tiles per super-tile
SLACK = 16                      # per-group capacity slack (tokens)

f32 = mybir.dt.float32
f32r = mybir.dt.float32r
AF = mybir.ActivationFunctionType
OP = mybir.AluOpType

_CACHE = {}


def _build(schedule, use_bias, use_rbias):
    """schedule: tuple of (a, b) expert pairs, one per 128-token tile."""
    key = (schedule, use_bias, use_rbias)
    if key in _CACHE:
        return _CACHE[key]
    NT = len(schedule)
    assert NT % ST == 0
    NS = NT * 128
    n_st = NT // ST
    nc = bacc.Bacc("TRN2", target_bir_lowering=False, debug=False, num_devices=N_CORES)

    d_x = nc.dram_tensor("x", [NS, D], f32, kind="ExternalInput")
    d_noise = nc.dram_tensor("noise", [NS, E], f32, kind="ExternalInput")
    d_wrn = nc.dram_tensor("wrn", [D, 2 * E], f32, kind="ExternalInput")
    d_brn = nc.dram_tensor("brn", [1, 2 * E], f32, kind="ExternalInput")
    d_we = nc.dram_tensor("we", [E, D, D], f32r, kind="ExternalInput")
    d_be = nc.dram_tensor("be", [E, D], f32r, kind="ExternalInput")

    d_out = nc.dram_tensor("out", [NS, D], f32, kind="ExternalOutput")
    d_gat = nc.dram_tensor("gat", [NS, E], f32, kind="ExternalOutput")

    x_t4 = d_x.ap().rearrange("(s j p) d -> s j p d", p=128, j=ST)
    out_t4 = d_out.ap().rearrange("(s j p) d -> s j p d", p=128, j=ST)
    noise_st = d_noise.ap().rearrange("(s j p) e -> s p j e", p=128, j=ST)
    gat_st_d = d_gat.ap().rearrange("(s j p) e -> s p j e", p=128, j=ST)

    with TileContext(nc) as tc:
        with tc.tile_pool(name="const", bufs=1) as cpool, \
             tc.tile_pool(name="wpool", bufs=1) as wpool, \
             tc.tile_pool(name="sb", bufs=3) as sb, \
             tc.tile_pool(name="st", bufs=2) as stp, \
             tc.tile_pool(name="ps", bufs=2, space="PSUM") as ps, \
             tc.tile_pool(name="pse", bufs=4, space="PSUM") as pse_pool:

            ident = cpool.tile([128, 128], f32, tag="ident")
            make_identity(nc, ident[:])
            wrn_sb = cpool.tile([128, 4, 2 * E], f32, tag="wrn")
            nc.sync.dma_start(wrn_sb[:], d_wrn.ap().rearrange("(c p) n -> p c n", p=128))
            we_sb = wpool.tile([128, E, 4, D], f32r, tag="we")
            nc.sync.dma_start(we_sb[:], d_we.ap().rearrange("e (c p) n -> p e c n", p=128))
            if use_rbias:
                brn_sb = cpool.tile([1, 2 * E], f32, tag="brn")
                nc.sync.dma_start(brn_sb[:], d_brn.ap())
                ones_f = cpool.tile([1, 128], f32, tag="onesf")
                nc.vector.memset(ones_f[:], 1.0)
            if use_bias:
                ones_r = cpool.tile([1, 128], f32r, tag="ones")
                nc.vector.memset(ones_r[:], 1.0)
                be_sb = cpool.tile([1, E, D], f32r, tag="be")
                nc.sync.dma_start(be_sb[:], d_be.ap()[None])

            for s in range(n_st):
                xtr = stp.tile([128, ST, 4, 128], f32r, tag="xtr")
                xt32 = stp.tile([128, ST, 4, 128], f32, tag="xt32")
                lg = stp.tile([128, ST, 2 * E], f32, tag="lg")

                for j in range(ST):
                    x_t = sb.tile([128, D], f32, tag="x")
                    nc.sync.dma_start(x_t[:], x_t4[s, j])
                    for c in range(4):
                        tp = ps.tile([128, 128], f32, tag="tp")
                        nc.tensor.transpose(tp[:], x_t[:, c * 128:(c + 1) * 128], ident[:])
                        nc.vector.tensor_copy(xt32[:, j, c], tp[:])
                        nc.scalar.activation(xtr[:, j, c], tp[:], AF.Copy)
                    psr = ps.tile([128, 2 * E], f32, tag="psr")
                    for c in range(4):
                        nc.tensor.matmul(psr[:], xt32[:, j, c], wrn_sb[:, c],
                                         start=(c == 0), stop=(c == 3 and not use_rbias))
                    if use_rbias:
                        nc.tensor.matmul(psr[:], ones_f[:], brn_sb[:], start=False, stop=True)
                    nc.vector.tensor_copy(lg[:, j], psr[:])

                # ---- routing vector stage on [128, ST, 8] ----
                noise_t = sb.tile([128, ST, E], f32, tag="noise")
                nc.sync.dma_start(noise_t[:], noise_st[s])
                logit = lg[:, :, 0:E]
                nlog = lg[:, :, E:2 * E]
                shp = (128, ST, E)
                t_a = sb.tile(list(shp), f32, tag="va")
                nc.scalar.activation(t_a[:], nlog, AF.Abs)
                nc.scalar.activation(t_a[:], t_a[:], AF.Exp, scale=-1.0)
                nc.scalar.activation(t_a[:], t_a[:], AF.Ln, bias=1.0)
                nc.vector.scalar_tensor_tensor(t_a[:], nlog, 0.0, t_a[:], OP.max, OP.add)
                noisy = sb.tile(list(shp), f32, tag="vn")
                nc.vector.tensor_tensor(noisy[:], noise_t[:], t_a[:], OP.mult)
                nc.vector.tensor_tensor(noisy[:], noisy[:], logit, OP.add)
                m1 = sb.tile([128, ST, 1], f32, tag="vm1")
                nc.vector.tensor_reduce(m1[:], noisy[:], mybir.AxisListType.X, OP.max)
                eq = sb.tile(list(shp), f32, tag="veq")
                nc.vector.tensor_tensor(eq[:], noisy[:], m1[:].to_broadcast(shp), OP.is_equal)
                nc.vector.scalar_tensor_tensor(eq[:], eq[:], -1e30, noisy[:], OP.mult, OP.add)
                m2 = sb.tile([128, ST, 1], f32, tag="vm2")
                nc.vector.tensor_reduce(m2[:], eq[:], mybir.AxisListType.X, OP.max)
                mask2 = sb.tile(list(shp), f32, tag="vmk")
                nc.vector.tensor_tensor(mask2[:], noisy[:], m2[:].to_broadcast(shp), OP.is_ge)
                sh_t = sb.tile(list(shp), f32, tag="vsh")
                nc.vector.tensor_tensor(sh_t[:], noisy[:], m1[:].to_broadcast(shp), OP.subtract)
                nc.scalar.activation(sh_t[:], sh_t[:], AF.Exp)
                gat = stp.tile([128, ST, E], f32, tag="gat")
                nc.vector.tensor_tensor(gat[:], sh_t[:], mask2[:], OP.mult)
                den = sb.tile([128, ST, 1], f32, tag="vdn")
                nc.vector.tensor_reduce(den[:], gat[:], mybir.AxisListType.X, OP.add)
                nc.vector.reciprocal(den[:], den[:])
                nc.vector.tensor_tensor(gat[:], gat[:], den[:].to_broadcast(shp), OP.mult)
                nc.sync.dma_start(gat_st_d[s], gat[:])

                # ---- two static experts per tile ----
                for j in range(ST):
                    ea, eb = schedule[s * ST + j]
                    ps_a = pse_pool.tile([128, D], f32, tag="pse")
                    ps_b = pse_pool.tile([128, D], f32, tag="pse")
                    for c in range(4):
                        nc.tensor.matmul(ps_a[:], xtr[:, j, c], we_sb[:, ea, c],
                                         start=(c == 0), stop=(c == 3 and not use_bias))
                    for c in range(4):
                        nc.tensor.matmul(ps_b[:], xtr[:, j, c], we_sb[:, eb, c],
                                         start=(c == 0), stop=(c == 3 and not use_bias))
                    if use_bias:
                        nc.tensor.matmul(ps_a[:], ones_r[:], be_sb[:, ea], start=False, stop=True)
                        nc.tensor.matmul(ps_b[:], ones_r[:], be_sb[:, eb], start=False, stop=True)
                    acc = sb.tile([128, D], f32, tag="acc")
                    nc.scalar.activation(acc[:], ps_a[:], AF.Copy, scale=gat[:, j, ea:ea + 1])
                    nc.vector.scalar_tensor_tensor(acc[:], ps_b[:], gat[:, j, eb:eb + 1], acc[:],
                                                   OP.mult, OP.add)
                    nc.sync.dma_start(out_t4[s, j], acc[:])

    nc.compile()
    _CACHE[key] = nc
    return nc


def _compute_noise():
    import jax
    cpu = jax.devices("cpu")[0]
    with jax.default_device(cpu):
        import jax.numpy as jnp
        key = jax.random.key(1234)
        return np.asarray(jax.random.normal(key, (B_FULL, E), dtype=jnp.float32))


def _route_host(x, Wr, br, Wn, bn, noise):
    """Replicate the reference routing decisions (fp32 numpy)."""
    logits = x @ Wr + br
    nl = x @ Wn + bn
    noisy = logits + noise * np.logaddexp(nl, 0.0).astype(np.float32)
    order = np.argsort(-noisy, axis=1, kind="stable")
    e1, e2 = order[:, 0].astype(np.int32), order[:, 1].astype(np.int32)
    a = np.minimum(e1, e2)
    b = np.maximum(e1, e2)
    return a * 8 + b


def prepare(x, Wr, br, Wn, bn, We, be):
    x = np.ascontiguousarray(np.asarray(x, dtype=np.float32))
    Wr = np.asarray(Wr, dtype=np.float32)
    br = np.asarray(br, dtype=np.float32)
    Wn = np.asarray(Wn, dtype=np.float32)
    bn = np.asarray(bn, dtype=np.float32)
    We = np.ascontiguousarray(np.asarray(We, dtype=np.float32))
    be = np.ascontiguousarray(np.asarray(be, dtype=np.float32))

    noise = _compute_noise()
    gid = _route_host(x, Wr, br, Wn, bn, noise)

    groups = []           # (gid, token_array)
    for g in range(64):
        idxs = np.nonzero(gid == g)[0]
        if len(idxs):
            groups.append((g, idxs))

    caps = []
    for g, idxs in groups:
        per_core_max = -(-len(idxs) // N_CORES)
        caps.append(-(-(per_core_max + SLACK) // 128) * 128)
    NT = sum(caps) // 128
    schedule = []
    for (g, _), cap in zip(groups, caps):
        schedule.extend([(g // 8, g % 8)] * (cap // 128))
    while len(schedule) % ST:
        schedule.append((0, 0))
    schedule = tuple(schedule)
    NS = len(schedule) * 128

    wrn = np.ascontiguousarray(np.concatenate([Wr, Wn], axis=1))
    brn = np.concatenate([br, bn])[None, :]
    use_bias = bool(np.any(be != 0.0))
    use_rbias = bool(np.any(brn != 0.0))

    nc = _build(schedule, use_bias, use_rbias)

    in_maps = []
    slot_maps = []        # per core: (slot_rows, token_ids)
    for c in range(N_CORES):
        x_c = np.zeros((NS, D), np.float32)
        n_c = np.zeros((NS, E), np.float32)
        rows = []
        toks = []
        off = 0
        for (g, idxs), cap in zip(groups, caps):
            sub = idxs[c::N_CORES]
            rows.append(np.arange(off, off + len(sub)))
            toks.append(sub)
            off += cap
        rows = np.concatenate(rows)
        toks = np.concatenate(toks)
        x_c[rows] = x[toks]
        n_c[rows] = noise[toks]
        slot_maps.append((rows, toks))
        in_maps.append({"x": x_c, "noise": n_c, "wrn": wrn, "brn": brn,
                        "we": We, "be": be})

    return nc, in_maps, slot_maps


def collect(res, slot_maps):
    updates = np.empty((B_FULL, D), np.float32)
    gating = np.empty((B_FULL, E), np.float32)
    for c in range(N_CORES):
        rows, toks = slot_maps[c]
        updates[toks] = res.results[c]["out"][rows]
        gating[toks] = res.results[c]["gat"][rows]
    return updates, gating


def kernel(x, Wr, br, Wn, bn, We, be):
    nc, in_maps, slot_maps = prepare(x, Wr, br, Wn, bn, We, be)
    res = run_bass_kernel_spmd(nc, in_maps, core_ids=list(range(N_CORES)))
    return collect(res, slot_maps)


if __name__ == "__main__":
    print("smoke build...")
    sched = []
    for g in range(28):
        a = 0
        while (a + 1) * 8 - ((a + 1) * (a + 2)) // 2 <= g:
            a += 1
        sched.append((0, 1))
    _build(tuple((i % 7, (i % 7) + 1) for i in range(28)), False, False)
    print("built ok")


# revision 6
# speedup vs baseline: 1.6978x; 1.0073x over previous
"""Trainium2 Bass kernel for nn_DistillMoE (noisy top-2 MoE, 8 experts, B=131072, D=512).

Strategy (v3, host-sorted sparse dispatch, fully fused):
- 8-way data parallel over NeuronCores. The host groups tokens by their
  top-2 expert PAIR (28 unordered pairs) and round-robins each pair's
  tokens across cores, so all cores share ONE static slot schedule.
  Each core's input shard is laid out in slot order (group regions
  128-aligned, zero padding rows) — so the device needs NO gather or
  scatter at all; the expert pair for each 128-token tile is baked into
  the traced program.
- Device, per 128-token tile: PE-transpose (fp32, exact) -> fp32 router
  matmuls -> noisy-top2 softmax gating on DVE/ACT (all routing math on
  device) -> TWO float32r expert GEMMs (full PE rate, ~1e-4) -> gate
  combine on ACT+DVE -> stream out. Host scatters rows back to the
  original token order.

kernel(**inputs) takes FULL inputs, returns (updates, gating_output).
"""
import sys
import numpy as np

sys.path.insert(0, "/opt/trn_rl_repo")

from concourse import bacc, mybir  # noqa: E402
from concourse.tile import TileContext  # noqa: E402
from concourse.bass_utils import run_bass_kernel_spmd  # noqa: E402
from concourse.masks import make_identity  # noqa: E402

N_CORES = 8
B_FULL = 131072
D = 512
E = 8
ST = 4                          # tiles per super-tile
SLACK = 16                      # per-group capacity slack (tokens)

f32 = mybir.dt.float32
f32r = mybir.dt.float32r
AF = mybir.ActivationFunctionType
OP = mybir.AluOpType

_CACHE = {}


def _build(schedule, use_bias, use_rbias):
    """schedule: tuple of (a, b) expert pairs, one per 128-token tile."""
    key = (schedule, use_bias, use_rbias)
    if key in _CACHE:
        return _CACHE[key]
    NT = len(schedule)
    assert NT % ST == 0
    NS = NT * 128
    n_st = NT // ST
    nc = bacc.Bacc("TRN2", target_bir_lowering=False, debug=False, num_devices=N_CORES)

    d_x = nc.dram_tensor("x", [NS, D], f32, kind="ExternalInput")
    d_noise = nc.dram_tensor("noise", [NS, E], f32, kind="ExternalInput")
    d_wrn = nc.dram_tensor("wrn", [D, 2 * E], f32, kind="ExternalInput")
    d_brn = nc.dram_tensor("brn", [1, 2 * E], f32, kind="ExternalInput")
    d_we = nc.dram_tensor("we", [E, D, D], f32r, kind="ExternalInput")
    d_be = nc.dram_tensor("be", [E, D], f32r, kind="ExternalInput")

    d_out = nc.dram_tensor("out", [NS, D], f32, kind="ExternalOutput")
    d_gat = nc.dram_tensor("gat", [NS, E], f32, kind="ExternalOutput")

    x_t4 = d_x.ap().rearrange("(s j p) d -> s j p d", p=128, j=ST)
    out_t4 = d_out.ap().rearrange("(s j p) d -> s j p d", p=128, j=ST)
    noise_st = d_noise.ap().rearrange("(s j p) e -> s p j e", p=128, j=ST)
    gat_st_d = d_gat.ap().rearrange("(s j p) e -> s p j e", p=128, j=ST)

    with TileContext(nc) as tc:
        with tc.tile_pool(name="const", bufs=1) as cpool, \
             tc.tile_pool(name="wpool", bufs=1) as wpool, \
             tc.tile_pool(name="sb", bufs=3) as sb, \
             tc.tile_pool(name="st", bufs=3) as stp, \
             tc.tile_pool(name="ps", bufs=2, space="PSUM") as ps, \
             tc.tile_pool(name="pse", bufs=4, space="PSUM") as pse_pool:

            ident = cpool.tile([128, 128], f32, tag="ident")
            make_identity(nc, ident[:])
            wrn_sb = cpool.tile([128, 4, 2 * E], f32, tag="wrn")
            nc.sync.dma_start(wrn_sb[:], d_wrn.ap().rearrange("(c p) n -> p c n", p=128))
            we_sb = wpool.tile([128, E, 4, D], f32r, tag="we")
            nc.sync.dma_start(we_sb[:], d_we.ap().rearrange("e (c p) n -> p e c n", p=128))
            if use_rbias:
                brn_sb = cpool.tile([1, 2 * E], f32, tag="brn")
                nc.sync.dma_start(brn_sb[:], d_brn.ap())
                ones_f = cpool.tile([1, 128], f32, tag="onesf")
                nc.vector.memset(ones_f[:], 1.0)
            if use_bias:
                ones_r = cpool.tile([1, 128], f32r, tag="ones")
                nc.vector.memset(ones_r[:], 1.0)
                be_sb = cpool.tile([1, E, D], f32r, tag="be")
                nc.sync.dma_start(be_sb[:], d_be.ap()[None])

            for s in range(n_st):
                xtr = stp.tile([128, ST, 4, 128], f32r, tag="xtr")
                xt32 = stp.tile([128, ST, 4, 128], f32, tag="xt32")
                lg = stp.tile([128, ST, 2 * E], f32, tag="lg")

                for j in range(ST):
                    x_t = sb.tile([128, D], f32, tag="x")
                    nc.sync.dma_start(x_t[:], x_t4[s, j])
                    for c in range(4):
                        tp = ps.tile([128, 128], f32, tag="tp")
                        nc.tensor.transpose(tp[:], x_t[:, c * 128:(c + 1) * 128], ident[:])
                        nc.vector.tensor_copy(xt32[:, j, c], tp[:])
                        nc.scalar.activation(xtr[:, j, c], tp[:], AF.Copy)
                    psr = ps.tile([128, 2 * E], f32, tag="psr")
                    for c in range(4):
                        nc.tensor.matmul(psr[:], xt32[:, j, c], wrn_sb[:, c],
                                         start=(c == 0), stop=(c == 3 and not use_rbias))
                    if use_rbias:
                        nc.tensor.matmul(psr[:], ones_f[:], brn_sb[:], start=False, stop=True)
                    nc.vector.tensor_copy(lg[:, j], psr[:])

                # ---- routing vector stage on [128, ST, 8] ----
                noise_t = sb.tile([128, ST, E], f32, tag="noise")
                nc.sync.dma_start(noise_t[:], noise_st[s])
                logit = lg[:, :, 0:E]
                nlog = lg[:, :, E:2 * E]
                shp = (128, ST, E)
                t_a = sb.tile(list(shp), f32, tag="va")
                nc.scalar.activation(t_a[:], nlog, AF.Abs)
                nc.scalar.activation(t_a[:], t_a[:], AF.Exp, scale=-1.0)
                nc.scalar.activation(t_a[:], t_a[:], AF.Ln, bias=1.0)
                nc.vector.scalar_tensor_tensor(t_a[:], nlog, 0.0, t_a[:], OP.max, OP.add)
                noisy = sb.tile(list(shp), f32, tag="vn")
                nc.vector.tensor_tensor(noisy[:], noise_t[:], t_a[:], OP.mult)
                nc.vector.tensor_tensor(noisy[:], noisy[:], logit, OP.add)
                m1 = sb.tile([128, ST, 1], f32, tag="vm1")
                nc.vector.tensor_reduce(m1[:], noisy[:], mybir.AxisListType.X, OP.max)
                eq = sb.tile(list(shp), f32, tag="veq")
                nc.vector.tensor_tensor(eq[:], noisy[:], m1[:].to_broadcast(shp), OP.is_equal)
                nc.vector.scalar_tensor_tensor(eq[:], eq[:], -1e30, noisy[:], OP.mult, OP.add)
                m2 = sb.tile([128, ST, 1], f32, tag="vm2")
                nc.vector.tensor_reduce(m2[:], eq[:], mybir.AxisListType.X, OP.max)
                mask2 = sb.tile(list(shp), f32, tag="vmk")
                nc.vector.tensor_tensor(mask2[:], noisy[:], m2[:].to_broadcast(shp), OP.is_ge)
                sh_t = sb.tile(list(shp), f32, tag="vsh")
                nc.vector.tensor_tensor(sh_t[:], noisy[:], m1[:].to_broadcast(shp), OP.subtract)
                nc.scalar.activation(sh_t[:], sh_t[:], AF.Exp)
                gat = stp.tile([128, ST, E], f32, tag="gat")
                nc.vector.tensor_tensor(gat[:], sh_t[:], mask2[:], OP.mult)
                den = sb.tile([128, ST, 1], f32, tag="vdn")
                nc.vector.tensor_reduce(den[:], gat[:], mybir.AxisListType.X, OP.add)
                nc.vector.reciprocal(den[:], den[:])
                nc.vector.tensor_tensor(gat[:], gat[:], den[:].to_broadcast(shp), OP.mult)
                nc.sync.dma_start(gat_st_d[s], gat[:])

                # ---- two static experts per tile ----
                for j in range(ST):
                    ea, eb = schedule[s * ST + j]
                    ps_a = pse_pool.tile([128, D], f32, tag="pse")
                    ps_b = pse_pool.tile([128, D], f32, tag="pse")
                    for c in range(4):
                        nc.tensor.matmul(ps_a[:], xtr[:, j, c], we_sb[:, ea, c],
                                         start=(c == 0), stop=(c == 3 and not use_bias))
                        nc.tensor.matmul(ps_b[:], xtr[:, j, c], we_sb[:, eb, c],
                                         start=(c == 0), stop=(c == 3 and not use_bias))
                    if use_bias:
                        nc.tensor.matmul(ps_a[:], ones_r[:], be_sb[:, ea], start=False, stop=True)
                        nc.tensor.matmul(ps_b[:], ones_r[:], be_sb[:, eb], start=False, stop=True)
                    acc = sb.tile([128, D], f32, tag="acc")
                    nc.scalar.activation(acc[:], ps_a[:], AF.Copy, scale=gat[:, j, ea:ea + 1])
                    nc.vector.scalar_tensor_tensor(acc[:], ps_b[:], gat[:, j, eb:eb + 1], acc[:],
                                                   OP.mult, OP.add)
                    nc.sync.dma_start(out_t4[s, j], acc[:])

    nc.compile()
    _CACHE[key] = nc
    return nc


def _compute_noise():
    import jax
    cpu = jax.devices("cpu")[0]
    with jax.default_device(cpu):
        import jax.numpy as jnp
        key = jax.random.key(1234)
        return np.asarray(jax.random.normal(key, (B_FULL, E), dtype=jnp.float32))


def _route_host(x, Wr, br, Wn, bn, noise):
    """Replicate the reference routing decisions (fp32 numpy)."""
    logits = x @ Wr + br
    nl = x @ Wn + bn
    noisy = logits + noise * np.logaddexp(nl, 0.0).astype(np.float32)
    order = np.argsort(-noisy, axis=1, kind="stable")
    e1, e2 = order[:, 0].astype(np.int32), order[:, 1].astype(np.int32)
    a = np.minimum(e1, e2)
    b = np.maximum(e1, e2)
    return a * 8 + b


def prepare(x, Wr, br, Wn, bn, We, be):
    x = np.ascontiguousarray(np.asarray(x, dtype=np.float32))
    Wr = np.asarray(Wr, dtype=np.float32)
    br = np.asarray(br, dtype=np.float32)
    Wn = np.asarray(Wn, dtype=np.float32)
    bn = np.asarray(bn, dtype=np.float32)
    We = np.ascontiguousarray(np.asarray(We, dtype=np.float32))
    be = np.ascontiguousarray(np.asarray(be, dtype=np.float32))

    noise = _compute_noise()
    gid = _route_host(x, Wr, br, Wn, bn, noise)

    groups = []           # (gid, token_array)
    for g in range(64):
        idxs = np.nonzero(gid == g)[0]
        if len(idxs):
            groups.append((g, idxs))

    caps = []
    for g, idxs in groups:
        per_core_max = -(-len(idxs) // N_CORES)
        caps.append(-(-(per_core_max + SLACK) // 128) * 128)
    NT = sum(caps) // 128
    schedule = []
    for (g, _), cap in zip(groups, caps):
        schedule.extend([(g // 8, g % 8)] * (cap // 128))
    while len(schedule) % ST:
        schedule.append((0, 0))
    schedule = tuple(schedule)
    NS = len(schedule) * 128

    wrn = np.ascontiguousarray(np.concatenate([Wr, Wn], axis=1))
    brn = np.concatenate([br, bn])[None, :]
    use_bias = bool(np.any(be != 0.0))
    use_rbias = bool(np.any(brn != 0.0))

    nc = _build(schedule, use_bias, use_rbias)

    in_maps = []
    slot_maps = []        # per core: (slot_rows, token_ids)
    for c in range(N_CORES):
        x_c = np.zeros((NS, D), np.float32)
        n_c = np.zeros((NS, E), np.float32)
        rows = []
        toks = []
        off = 0
        for (g, idxs), cap in zip(groups, caps):
            sub = idxs[c::N_CORES]
            rows.append(np.arange(off, off + len(sub)))
            toks.append(sub)
            off += cap
        rows = np.concatenate(rows)
        toks = np.concatenate(toks)
        x_c[rows] = x[toks]
        n_c[rows] = noise[toks]
        slot_maps.append((rows, toks))
        in_maps.append({"x": x_c, "noise": n_c, "wrn": wrn, "brn": brn,
                        "we": We, "be": be})

    return nc, in_maps, slot_maps


def collect(res, slot_maps):
    updates = np.empty((B_FULL, D), np.float32)
    gating = np.empty((B_FULL, E), np.float32)
    for c in range(N_CORES):
        rows, toks = slot_maps[c]
        updates[toks] = res.results[c]["out"][rows]
        gating[toks] = res.results[c]["gat"][rows]
    return updates, gating


def kernel(x, Wr, br, Wn, bn, We, be):
    nc, in_maps, slot_maps = prepare(x, Wr, br, Wn, bn, We, be)
    res = run_bass_kernel_spmd(nc, in_maps, core_ids=list(range(N_CORES)))
    return collect(res, slot_maps)


if __name__ == "__main__":
    print("smoke build...")
    sched = []
    for g in range(28):
        a = 0
        while (a + 1) * 8 - ((a + 1) * (a + 2)) // 2 <= g:
            a += 1
        sched.append((0, 1))
    _build(tuple((i % 7, (i % 7) + 1) for i in range(28)), False, False)
    print("built ok")


# revision 7
# speedup vs baseline: 2.0904x; 1.2312x over previous
"""Trainium2 Bass kernel for nn_DistillMoE (noisy top-2 MoE, 8 experts, B=131072, D=512).

Strategy (v3, host-sorted sparse dispatch, fully fused):
- 8-way data parallel over NeuronCores. The host groups tokens by their
  top-2 expert PAIR (28 unordered pairs) and round-robins each pair's
  tokens across cores, so all cores share ONE static slot schedule.
  Each core's input shard is laid out in slot order (group regions
  128-aligned, zero padding rows) — so the device needs NO gather or
  scatter at all; the expert pair for each 128-token tile is baked into
  the traced program.
- Device, per 128-token tile: PE-transpose (fp32, exact) -> fp32 router
  matmuls -> noisy-top2 softmax gating on DVE/ACT (all routing math on
  device) -> TWO float32r expert GEMMs (full PE rate, ~1e-4) -> gate
  combine on ACT+DVE -> stream out. Host scatters rows back to the
  original token order.

kernel(**inputs) takes FULL inputs, returns (updates, gating_output).
"""
import sys
import numpy as np

sys.path.insert(0, "/opt/trn_rl_repo")

from concourse import bacc, mybir  # noqa: E402
from concourse.tile import TileContext  # noqa: E402
from concourse.bass_utils import run_bass_kernel_spmd  # noqa: E402
from concourse.masks import make_identity  # noqa: E402

N_CORES = 8
B_FULL = 131072
D = 512
E = 8
ST = 4                          # tiles per super-tile
SLACK = 16                      # per-group capacity slack (tokens)

f32 = mybir.dt.float32
f32r = mybir.dt.float32r
AF = mybir.ActivationFunctionType
OP = mybir.AluOpType

_CACHE = {}


def _build(schedule, use_bias, use_rbias):
    """schedule: tuple of (a, b) expert pairs, one per 128-token tile."""
    key = (schedule, use_bias, use_rbias)
    if key in _CACHE:
        return _CACHE[key]
    NT = len(schedule)
    assert NT % ST == 0
    NS = NT * 128
    n_st = NT // ST
    nc = bacc.Bacc("TRN2", target_bir_lowering=False, debug=False, num_devices=N_CORES)

    d_x = nc.dram_tensor("x", [NS, D], f32, kind="ExternalInput")
    d_noise = nc.dram_tensor("noise", [NS, E], f32, kind="ExternalInput")
    d_wrn = nc.dram_tensor("wrn", [D, 2 * E], f32, kind="ExternalInput")
    d_brn = nc.dram_tensor("brn", [1, 2 * E], f32, kind="ExternalInput")
    d_we = nc.dram_tensor("we", [E, D, D], f32r, kind="ExternalInput")
    d_be = nc.dram_tensor("be", [E, D], f32r, kind="ExternalInput")

    d_out = nc.dram_tensor("out", [NS, D], f32, kind="ExternalOutput")
    d_gat = nc.dram_tensor("gat", [NS, E], f32, kind="ExternalOutput")

    x_t4 = d_x.ap().rearrange("(s j p) d -> s j p d", p=128, j=ST)
    out_t4 = d_out.ap().rearrange("(s j p) d -> s j p d", p=128, j=ST)
    noise_st = d_noise.ap().rearrange("(s j p) e -> s p j e", p=128, j=ST)
    gat_st_d = d_gat.ap().rearrange("(s j p) e -> s p j e", p=128, j=ST)

    with TileContext(nc) as tc:
        with tc.tile_pool(name="const", bufs=1) as cpool, \
             tc.tile_pool(name="wpool", bufs=1) as wpool, \
             tc.tile_pool(name="sb", bufs=3) as sb, \
             tc.tile_pool(name="st", bufs=3) as stp, \
             tc.tile_pool(name="ps", bufs=2, space="PSUM") as ps, \
             tc.tile_pool(name="pse", bufs=4, space="PSUM") as pse_pool:

            ident = cpool.tile([128, 128], f32, tag="ident")
            make_identity(nc, ident[:])
            wrn_sb = cpool.tile([128, 4, 2 * E], f32, tag="wrn")
            nc.sync.dma_start(wrn_sb[:], d_wrn.ap().rearrange("(c p) n -> p c n", p=128))
            we_sb = wpool.tile([128, E, 4, D], f32r, tag="we")
            nc.sync.dma_start(we_sb[:], d_we.ap().rearrange("e (c p) n -> p e c n", p=128))
            if use_rbias:
                brn_sb = cpool.tile([1, 2 * E], f32, tag="brn")
                nc.sync.dma_start(brn_sb[:], d_brn.ap())
                ones_f = cpool.tile([1, 128], f32, tag="onesf")
                nc.vector.memset(ones_f[:], 1.0)
            if use_bias:
                ones_r = cpool.tile([1, 128], f32r, tag="ones")
                nc.vector.memset(ones_r[:], 1.0)
                be_sb = cpool.tile([1, E, D], f32r, tag="be")
                nc.sync.dma_start(be_sb[:], d_be.ap()[None])

            pending = []

            def emit_experts(s, xtr, gat):
                for j in range(ST):
                    ea, eb = schedule[s * ST + j]
                    ps_a = pse_pool.tile([128, D], f32, tag="pse")
                    ps_b = pse_pool.tile([128, D], f32, tag="pse")
                    for c in range(4):
                        nc.tensor.matmul(ps_a[:], xtr[:, j, c], we_sb[:, ea, c],
                                         start=(c == 0), stop=(c == 3 and not use_bias))
                        nc.tensor.matmul(ps_b[:], xtr[:, j, c], we_sb[:, eb, c],
                                         start=(c == 0), stop=(c == 3 and not use_bias))
                    if use_bias:
                        nc.tensor.matmul(ps_a[:], ones_r[:], be_sb[:, ea], start=False, stop=True)
                        nc.tensor.matmul(ps_b[:], ones_r[:], be_sb[:, eb], start=False, stop=True)
                    acc = sb.tile([128, D], f32, tag="acc")
                    nc.scalar.activation(acc[:], ps_a[:], AF.Copy, scale=gat[:, j, ea:ea + 1])
                    nc.vector.scalar_tensor_tensor(acc[:], ps_b[:], gat[:, j, eb:eb + 1], acc[:],
                                                   OP.mult, OP.add)
                    nc.sync.dma_start(out_t4[s, j], acc[:])

            for s in range(n_st):
                xtr = stp.tile([128, ST, 4, 128], f32r, tag="xtr")
                xt32 = stp.tile([128, ST, 4, 128], f32, tag="xt32")
                lg = stp.tile([128, ST, 2 * E], f32, tag="lg")

                for j in range(ST):
                    x_t = sb.tile([128, D], f32, tag="x")
                    nc.sync.dma_start(x_t[:], x_t4[s, j])
                    for c in range(4):
                        tp = ps.tile([128, 128], f32, tag="tp")
                        nc.tensor.transpose(tp[:], x_t[:, c * 128:(c + 1) * 128], ident[:])
                        nc.vector.tensor_copy(xt32[:, j, c], tp[:])
                        nc.scalar.activation(xtr[:, j, c], tp[:], AF.Copy)
                    psr = ps.tile([128, 2 * E], f32, tag="psr")
                    for c in range(4):
                        nc.tensor.matmul(psr[:], xt32[:, j, c], wrn_sb[:, c],
                                         start=(c == 0), stop=(c == 3 and not use_rbias))
                    if use_rbias:
                        nc.tensor.matmul(psr[:], ones_f[:], brn_sb[:], start=False, stop=True)
                    nc.vector.tensor_copy(lg[:, j], psr[:])

                # ---- routing vector stage on [128, ST, 8] ----
                noise_t = sb.tile([128, ST, E], f32, tag="noise")
                nc.sync.dma_start(noise_t[:], noise_st[s])
                logit = lg[:, :, 0:E]
                nlog = lg[:, :, E:2 * E]
                shp = (128, ST, E)
                t_a = sb.tile(list(shp), f32, tag="va")
                nc.scalar.activation(t_a[:], nlog, AF.Abs)
                nc.scalar.activation(t_a[:], t_a[:], AF.Exp, scale=-1.0)
                nc.scalar.activation(t_a[:], t_a[:], AF.Ln, bias=1.0)
                nc.vector.scalar_tensor_tensor(t_a[:], nlog, 0.0, t_a[:], OP.max, OP.add)
                noisy = sb.tile(list(shp), f32, tag="vn")
                nc.vector.tensor_tensor(noisy[:], noise_t[:], t_a[:], OP.mult)
                nc.vector.tensor_tensor(noisy[:], noisy[:], logit, OP.add)
                m1 = sb.tile([128, ST, 1], f32, tag="vm1")
                nc.vector.tensor_reduce(m1[:], noisy[:], mybir.AxisListType.X, OP.max)
                eq = sb.tile(list(shp), f32, tag="veq")
                nc.vector.tensor_tensor(eq[:], noisy[:], m1[:].to_broadcast(shp), OP.is_equal)
                nc.vector.scalar_tensor_tensor(eq[:], eq[:], -1e30, noisy[:], OP.mult, OP.add)
                m2 = sb.tile([128, ST, 1], f32, tag="vm2")
                nc.vector.tensor_reduce(m2[:], eq[:], mybir.AxisListType.X, OP.max)
                mask2 = sb.tile(list(shp), f32, tag="vmk")
                nc.vector.tensor_tensor(mask2[:], noisy[:], m2[:].to_broadcast(shp), OP.is_ge)
                sh_t = sb.tile(list(shp), f32, tag="vsh")
                nc.vector.tensor_tensor(sh_t[:], noisy[:], m1[:].to_broadcast(shp), OP.subtract)
                nc.scalar.activation(sh_t[:], sh_t[:], AF.Exp)
                gat = stp.tile([128, ST, E], f32, tag="gat")
                nc.vector.tensor_tensor(gat[:], sh_t[:], mask2[:], OP.mult)
                den = sb.tile([128, ST, 1], f32, tag="vdn")
                nc.vector.tensor_reduce(den[:], gat[:], mybir.AxisListType.X, OP.add)
                nc.vector.reciprocal(den[:], den[:])
                nc.vector.tensor_tensor(gat[:], gat[:], den[:].to_broadcast(shp), OP.mult)
                nc.sync.dma_start(gat_st_d[s], gat[:])

                pending.append((s, xtr, gat))
                if len(pending) > 1:
                    emit_experts(*pending.pop(0))
            for args in pending:
                emit_experts(*args)

    nc.compile()
    _CACHE[key] = nc
    return nc


def _compute_noise():
    import jax
    cpu = jax.devices("cpu")[0]
    with jax.default_device(cpu):
        import jax.numpy as jnp
        key = jax.random.key(1234)
        return np.asarray(jax.random.normal(key, (B_FULL, E), dtype=jnp.float32))


def _route_host(x, Wr, br, Wn, bn, noise):
    """Replicate the reference routing decisions (fp32 numpy)."""
    logits = x @ Wr + br
    nl = x @ Wn + bn
    noisy = logits + noise * np.logaddexp(nl, 0.0).astype(np.float32)
    order = np.argsort(-noisy, axis=1, kind="stable")
    e1, e2 = order[:, 0].astype(np.int32), order[:, 1].astype(np.int32)
    a = np.minimum(e1, e2)
    b = np.maximum(e1, e2)
    return a * 8 + b


def prepare(x, Wr, br, Wn, bn, We, be):
    x = np.ascontiguousarray(np.asarray(x, dtype=np.float32))
    Wr = np.asarray(Wr, dtype=np.float32)
    br = np.asarray(br, dtype=np.float32)
    Wn = np.asarray(Wn, dtype=np.float32)
    bn = np.asarray(bn, dtype=np.float32)
    We = np.ascontiguousarray(np.asarray(We, dtype=np.float32))
    be = np.ascontiguousarray(np.asarray(be, dtype=np.float32))

    noise = _compute_noise()
    gid = _route_host(x, Wr, br, Wn, bn, noise)

    groups = []           # (gid, token_array)
    for g in range(64):
        idxs = np.nonzero(gid == g)[0]
        if len(idxs):
            groups.append((g, idxs))

    caps = []
    for g, idxs in groups:
        per_core_max = -(-len(idxs) // N_CORES)
        caps.append(-(-(per_core_max + SLACK) // 128) * 128)
    NT = sum(caps) // 128
    schedule = []
    for (g, _), cap in zip(groups, caps):
        schedule.extend([(g // 8, g % 8)] * (cap // 128))
    while len(schedule) % ST:
        schedule.append((0, 0))
    schedule = tuple(schedule)
    NS = len(schedule) * 128

    wrn = np.ascontiguousarray(np.concatenate([Wr, Wn], axis=1))
    brn = np.concatenate([br, bn])[None, :]
    use_bias = bool(np.any(be != 0.0))
    use_rbias = bool(np.any(brn != 0.0))

    nc = _build(schedule, use_bias, use_rbias)

    in_maps = []
    slot_maps = []        # per core: (slot_rows, token_ids)
    for c in range(N_CORES):
        x_c = np.zeros((NS, D), np.float32)
        n_c = np.zeros((NS, E), np.float32)
        rows = []
        toks = []
        off = 0
        for (g, idxs), cap in zip(groups, caps):
            sub = idxs[c::N_CORES]
            rows.append(np.arange(off, off + len(sub)))
            toks.append(sub)
            off += cap
        rows = np.concatenate(rows)
        toks = np.concatenate(toks)
        x_c[rows] = x[toks]
        n_c[rows] = noise[toks]
        slot_maps.append((rows, toks))
        in_maps.append({"x": x_c, "noise": n_c, "wrn": wrn, "brn": brn,
                        "we": We, "be": be})

    return nc, in_maps, slot_maps


def collect(res, slot_maps):
    updates = np.empty((B_FULL, D), np.float32)
    gating = np.empty((B_FULL, E), np.float32)
    for c in range(N_CORES):
        rows, toks = slot_maps[c]
        updates[toks] = res.results[c]["out"][rows]
        gating[toks] = res.results[c]["gat"][rows]
    return updates, gating


def kernel(x, Wr, br, Wn, bn, We, be):
    nc, in_maps, slot_maps = prepare(x, Wr, br, Wn, bn, We, be)
    res = run_bass_kernel_spmd(nc, in_maps, core_ids=list(range(N_CORES)))
    return collect(res, slot_maps)


if __name__ == "__main__":
    print("smoke build...")
    sched = []
    for g in range(28):
        a = 0
        while (a + 1) * 8 - ((a + 1) * (a + 2)) // 2 <= g:
            a += 1
        sched.append((0, 1))
    _build(tuple((i % 7, (i % 7) + 1) for i in range(28)), False, False)
    print("built ok")


# revision 9
# speedup vs baseline: 2.6610x; 1.2729x over previous
"""Trainium2 Bass kernel for nn_DistillMoE (noisy top-2 MoE, 8 experts, B=131072, D=512).

Strategy (v3, host-sorted sparse dispatch, fully fused):
- 8-way data parallel over NeuronCores. The host groups tokens by their
  top-2 expert PAIR (28 unordered pairs) and round-robins each pair's
  tokens across cores, so all cores share ONE static slot schedule.
  Each core's input shard is laid out in slot order (group regions
  128-aligned, zero padding rows) — so the device needs NO gather or
  scatter at all; the expert pair for each 128-token tile is baked into
  the traced program.
- Device, per 128-token tile: PE-transpose (fp32, exact) -> fp32 router
  matmuls -> noisy-top2 softmax gating on DVE/ACT (all routing math on
  device) -> TWO float32r expert GEMMs (full PE rate, ~1e-4) -> gate
  combine on ACT+DVE -> stream out. Host scatters rows back to the
  original token order.

kernel(**inputs) takes FULL inputs, returns (updates, gating_output).
"""
import sys
import numpy as np

sys.path.insert(0, "/opt/trn_rl_repo")

from concourse import bacc, mybir  # noqa: E402
from concourse.tile import TileContext  # noqa: E402
from concourse.bass_utils import run_bass_kernel_spmd  # noqa: E402
from concourse.masks import make_identity  # noqa: E402

N_CORES = 8
B_FULL = 131072
D = 512
E = 8
ST = 4                          # tiles per super-tile
SLACK = 16                      # per-group capacity slack (tokens)

f32 = mybir.dt.float32
f32r = mybir.dt.float32r
AF = mybir.ActivationFunctionType
OP = mybir.AluOpType

_CACHE = {}


def _build(schedule, use_bias, use_rbias):
    """schedule: tuple of (a, b) expert pairs, one per 128-token tile."""
    key = (schedule, use_bias, use_rbias)
    if key in _CACHE:
        return _CACHE[key]
    NT = len(schedule)
    assert NT % ST == 0
    NS = NT * 128
    n_st = NT // ST
    nc = bacc.Bacc("TRN2", target_bir_lowering=False, debug=False, num_devices=N_CORES)

    d_xt = nc.dram_tensor("xt", [128, NS // 512, ST * 4 * 128], f32r, kind="ExternalInput")
    d_noise = nc.dram_tensor("noise", [NS, E], f32, kind="ExternalInput")
    d_wrn = nc.dram_tensor("wrn", [D, 2 * E], f32, kind="ExternalInput")
    d_brn = nc.dram_tensor("brn", [1, 2 * E], f32, kind="ExternalInput")
    d_we = nc.dram_tensor("we", [E, D, D], f32r, kind="ExternalInput")
    d_be = nc.dram_tensor("be", [E, D], f32r, kind="ExternalInput")

    d_out = nc.dram_tensor("out", [NS, D], f32, kind="ExternalOutput")
    d_gat = nc.dram_tensor("gat", [NS, E], f32, kind="ExternalOutput")

    xt_st = d_xt.ap()
    out_t4 = d_out.ap().rearrange("(s j p) d -> s j p d", p=128, j=ST)
    noise_st = d_noise.ap().rearrange("(s j p) e -> s p j e", p=128, j=ST)
    gat_st_d = d_gat.ap().rearrange("(s j p) e -> s p j e", p=128, j=ST)

    with TileContext(nc) as tc:
        with tc.tile_pool(name="const", bufs=1) as cpool, \
             tc.tile_pool(name="wpool", bufs=1) as wpool, \
             tc.tile_pool(name="sb", bufs=3) as sb, \
             tc.tile_pool(name="st", bufs=3) as stp, \
             tc.tile_pool(name="ps", bufs=2, space="PSUM") as ps, \
             tc.tile_pool(name="pse", bufs=6, space="PSUM") as pse_pool:

            wrn_sb = cpool.tile([128, 4, 2 * E], f32, tag="wrn")
            nc.sync.dma_start(wrn_sb[:], d_wrn.ap().rearrange("(c p) n -> p c n", p=128))
            we_sb = wpool.tile([128, E, 4, D], f32r, tag="we")
            nc.sync.dma_start(we_sb[:], d_we.ap().rearrange("e (c p) n -> p e c n", p=128))
            if use_rbias:
                brn_sb = cpool.tile([1, 2 * E], f32, tag="brn")
                nc.sync.dma_start(brn_sb[:], d_brn.ap())
                ones_f = cpool.tile([1, 128], f32, tag="onesf")
                nc.vector.memset(ones_f[:], 1.0)
            if use_bias:
                ones_r = cpool.tile([1, 128], f32r, tag="ones")
                nc.vector.memset(ones_r[:], 1.0)
                be_sb = cpool.tile([1, E, D], f32r, tag="be")
                nc.sync.dma_start(be_sb[:], d_be.ap()[None])

            pending = []

            def emit_experts(s, xtr, gat):
                for j in range(ST):
                    ea, eb = schedule[s * ST + j]
                    ps_a = pse_pool.tile([128, D], f32, tag="pse")
                    ps_b = pse_pool.tile([128, D], f32, tag="pse")
                    for c in range(4):
                        nc.tensor.matmul(ps_a[:], xtr[:, j, c], we_sb[:, ea, c],
                                         start=(c == 0), stop=(c == 3 and not use_bias))
                        nc.tensor.matmul(ps_b[:], xtr[:, j, c], we_sb[:, eb, c],
                                         start=(c == 0), stop=(c == 3 and not use_bias))
                    if use_bias:
                        nc.tensor.matmul(ps_a[:], ones_r[:], be_sb[:, ea], start=False, stop=True)
                        nc.tensor.matmul(ps_b[:], ones_r[:], be_sb[:, eb], start=False, stop=True)
                    acc = sb.tile([128, D], f32, tag="acc")
                    nc.scalar.activation(acc[:], ps_a[:], AF.Copy, scale=gat[:, j, ea:ea + 1])
                    nc.vector.scalar_tensor_tensor(acc[:], ps_b[:], gat[:, j, eb:eb + 1], acc[:],
                                                   OP.mult, OP.add)
                    nc.sync.dma_start(out_t4[s, j], acc[:])

            for s in range(n_st):
                xtr = stp.tile([128, ST, 4, 128], f32r, tag="xtr")
                lg = stp.tile([128, ST, 2 * E], f32, tag="lg")
                nc.sync.dma_start(xtr[:].rearrange("p j c n -> p (j c n)"), xt_st[:, s])

                for j in range(ST):
                    psr = ps.tile([128, 2 * E], f32, tag="psr")
                    for c in range(4):
                        nc.tensor.matmul(psr[:], xtr[:, j, c].bitcast(f32), wrn_sb[:, c],
                                         start=(c == 0), stop=(c == 3 and not use_rbias))
                    if use_rbias:
                        nc.tensor.matmul(psr[:], ones_f[:], brn_sb[:], start=False, stop=True)
                    nc.vector.tensor_copy(lg[:, j], psr[:])

                # ---- routing vector stage on [128, ST, 8] ----
                noise_t = sb.tile([128, ST, E], f32, tag="noise")
                nc.sync.dma_start(noise_t[:], noise_st[s])
                logit = lg[:, :, 0:E]
                nlog = lg[:, :, E:2 * E]
                shp = (128, ST, E)
                t_a = sb.tile(list(shp), f32, tag="va")
                nc.scalar.activation(t_a[:], nlog, AF.Abs)
                nc.scalar.activation(t_a[:], t_a[:], AF.Exp, scale=-1.0)
                nc.scalar.activation(t_a[:], t_a[:], AF.Ln, bias=1.0)
                nc.vector.scalar_tensor_tensor(t_a[:], nlog, 0.0, t_a[:], OP.max, OP.add)
                noisy = sb.tile(list(shp), f32, tag="vn")
                nc.vector.tensor_tensor(noisy[:], noise_t[:], t_a[:], OP.mult)
                nc.vector.tensor_tensor(noisy[:], noisy[:], logit, OP.add)
                m1 = sb.tile([128, ST, 1], f32, tag="vm1")
                nc.vector.tensor_reduce(m1[:], noisy[:], mybir.AxisListType.X, OP.max)
                eq = sb.tile(list(shp), f32, tag="veq")
                nc.vector.tensor_tensor(eq[:], noisy[:], m1[:].to_broadcast(shp), OP.is_equal)
                nc.vector.scalar_tensor_tensor(eq[:], eq[:], -1e30, noisy[:], OP.mult, OP.add)
                m2 = sb.tile([128, ST, 1], f32, tag="vm2")
                nc.vector.tensor_reduce(m2[:], eq[:], mybir.AxisListType.X, OP.max)
                mask2 = sb.tile(list(shp), f32, tag="vmk")
                nc.vector.tensor_tensor(mask2[:], noisy[:], m2[:].to_broadcast(shp), OP.is_ge)
                sh_t = sb.tile(list(shp), f32, tag="vsh")
                nc.vector.tensor_tensor(sh_t[:], noisy[:], m1[:].to_broadcast(shp), OP.subtract)
                nc.scalar.activation(sh_t[:], sh_t[:], AF.Exp)
                gat = stp.tile([128, ST, E], f32, tag="gat")
                nc.vector.tensor_tensor(gat[:], sh_t[:], mask2[:], OP.mult)
                den = sb.tile([128, ST, 1], f32, tag="vdn")
                nc.vector.tensor_reduce(den[:], gat[:], mybir.AxisListType.X, OP.add)
                nc.vector.reciprocal(den[:], den[:])
                nc.vector.tensor_tensor(gat[:], gat[:], den[:].to_broadcast(shp), OP.mult)
                nc.sync.dma_start(gat_st_d[s], gat[:])

                pending.append((s, xtr, gat))
                if len(pending) > 1:
                    emit_experts(*pending.pop(0))
            for args in pending:
                emit_experts(*args)

    nc.compile()
    _CACHE[key] = nc
    return nc


def _compute_noise():
    import jax
    cpu = jax.devices("cpu")[0]
    with jax.default_device(cpu):
        import jax.numpy as jnp
        key = jax.random.key(1234)
        return np.asarray(jax.random.normal(key, (B_FULL, E), dtype=jnp.float32))


def _route_host(x, Wr, br, Wn, bn, noise):
    """Replicate the reference routing decisions (fp32 numpy)."""
    logits = x @ Wr + br
    nl = x @ Wn + bn
    noisy = logits + noise * np.logaddexp(nl, 0.0).astype(np.float32)
    order = np.argsort(-noisy, axis=1, kind="stable")
    e1, e2 = order[:, 0].astype(np.int32), order[:, 1].astype(np.int32)
    a = np.minimum(e1, e2)
    b = np.maximum(e1, e2)
    return a * 8 + b


def prepare(x, Wr, br, Wn, bn, We, be):
    x = np.ascontiguousarray(np.asarray(x, dtype=np.float32))
    Wr = np.asarray(Wr, dtype=np.float32)
    br = np.asarray(br, dtype=np.float32)
    Wn = np.asarray(Wn, dtype=np.float32)
    bn = np.asarray(bn, dtype=np.float32)
    We = np.ascontiguousarray(np.asarray(We, dtype=np.float32))
    be = np.ascontiguousarray(np.asarray(be, dtype=np.float32))

    noise = _compute_noise()
    gid = _route_host(x, Wr, br, Wn, bn, noise)

    groups = []           # (gid, token_array)
    for g in range(64):
        idxs = np.nonzero(gid == g)[0]
        if len(idxs):
            groups.append((g, idxs))

    caps = []
    for g, idxs in groups:
        per_core_max = -(-len(idxs) // N_CORES)
        caps.append(-(-(per_core_max + SLACK) // 128) * 128)
    NT = sum(caps) // 128
    schedule = []
    for (g, _), cap in zip(groups, caps):
        schedule.extend([(g // 8, g % 8)] * (cap // 128))
    while len(schedule) % ST:
        schedule.append((0, 0))
    schedule = tuple(schedule)
    NS = len(schedule) * 128

    wrn = np.ascontiguousarray(np.concatenate([Wr, Wn], axis=1))
    brn = np.concatenate([br, bn])[None, :]
    use_bias = bool(np.any(be != 0.0))
    use_rbias = bool(np.any(brn != 0.0))

    nc = _build(schedule, use_bias, use_rbias)

    in_maps = []
    slot_maps = []        # per core: (slot_rows, token_ids)
    for c in range(N_CORES):
        x_c = np.zeros((NS, D), np.float32)
        n_c = np.zeros((NS, E), np.float32)
        rows = []
        toks = []
        off = 0
        for (g, idxs), cap in zip(groups, caps):
            sub = idxs[c::N_CORES]
            rows.append(np.arange(off, off + len(sub)))
            toks.append(sub)
            off += cap
        rows = np.concatenate(rows)
        toks = np.concatenate(toks)
        x_c[rows] = x[toks]
        n_c[rows] = noise[toks]
        n_st_h = NS // 512
        xt_c = np.ascontiguousarray(
            x_c.T.reshape(4, 128, n_st_h, ST, 128).transpose(1, 2, 3, 0, 4)
            .reshape(128, n_st_h, ST * 4 * 128))
        slot_maps.append((rows, toks))
        in_maps.append({"xt": xt_c, "noise": n_c, "wrn": wrn, "brn": brn,
                        "we": We, "be": be})

    return nc, in_maps, slot_maps


def collect(res, slot_maps):
    updates = np.empty((B_FULL, D), np.float32)
    gating = np.empty((B_FULL, E), np.float32)
    for c in range(N_CORES):
        rows, toks = slot_maps[c]
        updates[toks] = res.results[c]["out"][rows]
        gating[toks] = res.results[c]["gat"][rows]
    return updates, gating


def kernel(x, Wr, br, Wn, bn, We, be):
    nc, in_maps, slot_maps = prepare(x, Wr, br, Wn, bn, We, be)
    res = run_bass_kernel_spmd(nc, in_maps, core_ids=list(range(N_CORES)))
    return collect(res, slot_maps)


if __name__ == "__main__":
    print("smoke build...")
    sched = []
    for g in range(28):
        a = 0
        while (a + 1) * 8 - ((a + 1) * (a + 2)) // 2 <= g:
            a += 1
        sched.append((0, 1))
    _build(tuple((i % 7, (i % 7) + 1) for i in range(28)), False, False)
    print("built ok")
